# revision 35
# baseline (speedup 1.0000x reference)
"""Additive attention (Bahdanau) on 8 TRN2 NeuronCores.

Full-problem shapes: query [4,512,512], key/value [4,512,512],
Wq/Wk [512,256], bq/bk [256], wv [256], bv [].

  q = query @ Wq + bq                       # [B,Q,H]
  k = key @ Wk + bk                         # [B,K,H]
  score[b,q,k] = wv . tanh(q[b,q]+k[b,k])   # (+bv, dropped: softmax-invariant)
  attn = softmax(score, axis=-1)
  context = attn @ value

Sharding: data-parallel over (batch, query-half): core c handles batch c//2,
query rows (c%2)*256:(c%2+1)*256. Each core sees its full key/value batch, so
softmax is core-local. All inputs are shipped pre-permuted so every DMA is a
fully-contiguous [128, N] transfer; derived constants (activation arg biases,
a0*wv) are computed on the host.

Algorithm: the O(Q*K*H) tanh is replaced by a separable odd-harmonic sinusoid
expansion  tanh(x) ~= sum_j a_j sin((2j+1) w0 x),  x = q_h + k_h, so the score
becomes 2M h-contracting matmuls per key chunk on the tensor engine.  Base
sin/cos at w0 come off the scalar engine straight from the projection PSUM
(projection bias folded into the activation bias; cos via sin(pi/2 - t)).
Higher odd harmonics use the Chebyshev recurrence f_{m+2} = 2cos(2w0x) f_m -
f_{m-2} in fp16: the m=3 step is a single fused scalar_tensor_tensor
(sin3=(t+1)sin1, cos3=(t-1)cos1); k-side steps emit the cos half first (the
score matmuls consume cos as the stationary operand first); the q-side j=1/j=2
steps and nothing else run on the otherwise-idle GPSIMD engine. a0*wv folds
into the q-side base (tensor_scalar with a per-partition scalar, 4x mode);
per-harmonic a_j/a0 rescales are 4x-mode tensor_scalars on the DVE.

Tail: per key chunk a single bf16 exp (no max-subtraction; |score| <= ~13)
feeds (a) rank-1 PE row-sum accumulation, (b) the context matmuls in a
transposed [dv,q] layout (v-slice stationary), and (c) the attn normalize.
One reciprocal of the row sums is broadcast across partitions via a rank-1 PE
outer product, copied to SBUF fp16, and both attn and ctxT are normalized by
it on the DVE in 16-bit (2x) and DMA'd out as bf16; the host transposes and
upcasts.  PE clock is kept warm from t=0 by a dummy-matmul stream sized to
cover the input-DMA window.
"""

import ml_dtypes
import numpy as np

import concourse.bass as bass
import concourse.tile as tile
from concourse import bacc, mybir
from concourse.bass_utils import run_bass_kernel_spmd

F32 = mybir.dt.float32
F16 = mybir.dt.float16
BF16 = mybir.dt.bfloat16
AF = mybir.ActivationFunctionType
ALU = mybir.AluOpType

P = 128          # partitions
D = 512          # DQ = DK (projection input dim)
H = 256          # hidden dim; HC = H // P h-chunks
K = 512          # keys per batch; KC = K // P key chunks
QS = 256         # query rows per core
DV = 512         # value dim
HC, KC, DC, QT = H // P, K // P, D // P, QS // P

N_CORES = 8
B, Q = 4, 512

HALF_PI = float(np.pi / 2)

# odd-harmonic fit of tanh on [-9.5, 9.5]: tanh(x) ~ sum a_j sin((2j+1) OM0 x)
M = 4
OM0 = 0.332800
A_COEF = [1.2113965, 0.29704528, 0.088363231, 0.047231174]

SIN, COS = 0, 1


def _build_tile_kernel(tc, ins, outs):
    nc = tc.nc
    qT_in, kT_in, v_in, wq_in, wk_in, consts_in = ins
    attnT_out, ctxT_out = outs

    with tc.tile_pool(name="const", bufs=1) as const, \
         tc.tile_pool(name="proj", bufs=1) as proj, \
         tc.tile_pool(name="chain", bufs=1) as chain, \
         tc.tile_pool(name="scr", bufs=1) as scr, \
         tc.tile_pool(name="tailp", bufs=1) as tailp:

        # ---- input DMAs: every transfer is contiguous [128, N].  q path
        #      first (its sins gate the whole feature chain).  Weights go
        #      out on the DVE queue: a DMA desc-gen on the SCALAR queue
        #      between an ACT table load and its activations forces a
        #      ~1.3us table RELOAD (observed), so scalar issues no DMAs
        #      before the sins.  v is deferred to after the sins. ---------
        warm = const.tile([P, 256], F16)
        nc.vector.memset(warm[:], 0.25)
        qT = proj.tile([P, DC, QS], F16)     # [p, d_chunk, q]
        nc.sync.dma_start(qT[:], qT_in)
        wq16 = proj.tile([P, DC, H], F16)
        nc.sync.dma_start(wq16[:], wq_in)
        kT = proj.tile([P, DC, K], F16)
        nc.sync.dma_start(kT[:], kT_in)
        wk16 = proj.tile([P, DC, H], F16)
        nc.sync.dma_start(wk16[:], wk_in)
        # consts rows: 0 OM0*bq, 1 pi/2-OM0*bq, 2 OM0*bk, 3 pi/2-OM0*bk, 4 a0*wv
        consts = const.tile([P, 5, HC], F32)
        nc.gpsimd.dma_start(consts[:], consts_in)
        ones_bf = const.tile([P, 1], BF16)   # k-sum matmul lhsT
        nc.gpsimd.memset(ones_bf[:], 1.0)
        ones16 = const.tile([1, P], F16)     # partition-broadcast via PE
        nc.gpsimd.memset(ones16[:], 1.0)
        # dummy Sin pulls the trig table load off the critical path
        warm_sin = const.tile([P, 1], F16)
        nc.scalar.activation(warm_sin[:], warm[:, 0:1], AF.Sin)

        # feature tiles: [:, 0]=sin half, [:, 1]=cos half (ek[3] is never
        # materialized: its multiply feeds the score directly per chunk)
        scq = chain.tile([P, 2, HC, QS], F16)   # unfolded q base sin/cos
        ek = [chain.tile([P, 2, HC, K], F16, name=f"ek{j}") for j in range(M - 1)]
        eq = [chain.tile([P, 2, HC, QS], F16, name=f"eq{j}") for j in range(M)]
        aq = [chain.tile([P, 2, HC, QS], F16, name=f"aq{j}") for j in range(M)]

        with tc.tile_pool(name="ps_proj", bufs=1, space="PSUM") as ps_proj:
            wps = ps_proj.tile([P, 256], F32, tag="warm", bufs=1)
            for _ in range(20):
                nc.tensor.matmul(wps[:], warm[:, 0:P], warm[:],
                                 start=True, stop=True)
            # -- query path first: project -> sins ------------------------
            psqs = []
            for hs in range(HC):
                psq = ps_proj.tile([P, QS], F32, tag=f"psq{hs}", bufs=1,
                                   name=f"psq{hs}")
                for c in range(DC):
                    nc.tensor.matmul(psq[:], wq16[:, c, hs * P:(hs + 1) * P],
                                     qT[:, c, :], start=(c == 0), stop=(c == DC - 1))
                psqs.append(psq)
            for hs in range(HC):
                nc.scalar.activation(scq[:, SIN, hs, :], psqs[hs][:], AF.Sin,
                                     bias=consts[:, 0, hs:hs + 1], scale=OM0)
            for hs in range(HC):
                nc.scalar.activation(scq[:, COS, hs, :], psqs[hs][:], AF.Sin,
                                     bias=consts[:, 1, hs:hs + 1], scale=-OM0)
            # -- key path: project -> sins (into stacked e0) --------------
            psks = []
            for hs in range(HC):
                psk = ps_proj.tile([P, K], F32, tag=f"psk{hs}", bufs=1,
                                   name=f"psk{hs}")
                for c in range(DC):
                    nc.tensor.matmul(psk[:], wk16[:, c, hs * P:(hs + 1) * P],
                                     kT[:, c, :], start=(c == 0), stop=(c == DC - 1))
                psks.append(psk)
            for hs in range(HC):
                nc.scalar.activation(ek[0][:, SIN, hs, :], psks[hs][:], AF.Sin,
                                     bias=consts[:, 2, hs:hs + 1], scale=OM0)
            for hs in range(HC):
                nc.scalar.activation(ek[0][:, COS, hs, :], psks[hs][:], AF.Sin,
                                     bias=consts[:, 3, hs:hs + 1], scale=-OM0)

            # -- chains, all on the DVE (gpsimd elementwise is ~4x slower
            #    AND steals the DVE's SBUF port — measured net negative).
            #    Ops are stacked [P,2,HC,L] wherever the two halves share
            #    an ALU op (one ~146ns issue overhead instead of two);
            #    j=1 is a fused STT per half: sin3=(t+1)sin1, cos3=(t-1)cos1.
            #    The k j=3 SUBTRACT is absorbed into the score matmuls
            #    (score_3 = aq3 x (tk*ek2)  -  aq3 x ek1, via a negated
            #    fold), so only the j=3 multiply remains on the DVE, and it
            #    is split per key-chunk so the score/exp tail pipelines
            #    into the end of the chain. ------------------------------
            tq = chain.tile([P, HC, QS], F16)
            tmp = scr.tile([P, HC, QS], F16, tag="tbq")
            nc.vector.tensor_tensor(tmp[:], scq[:, SIN], scq[:, SIN], ALU.mult)
            nc.vector.tensor_scalar(tq[:], tmp[:], -4.0, 2.0, ALU.mult, ALU.add)
            for hs in range(HC):
                nc.vector.tensor_scalar_mul(eq[0][:, :, hs, :], scq[:, :, hs, :],
                                            consts[:, 4, hs:hs + 1])
            aq[0] = eq[0]

            tqb = tq[:, None, :, :].to_broadcast((P, 1, HC, QS))
            tqb2 = tq[:, None, :, :].to_broadcast((P, 2, HC, QS))
            nc.vector.scalar_tensor_tensor(eq[1][:, SIN:SIN + 1], tqb, 1.0,
                                           eq[0][:, SIN:SIN + 1], ALU.add, ALU.mult)
            nc.vector.scalar_tensor_tensor(eq[1][:, COS:COS + 1], tqb, -1.0,
                                           eq[0][:, COS:COS + 1], ALU.add, ALU.mult)

            def fold(j):
                nc.vector.tensor_scalar_mul(aq[j][:], eq[j][:],
                                            float(A_COEF[j] / A_COEF[0]))

            fold(1)

            sqk = scr.tile([P, HC, K], F16, tag="tbk")
            tk = chain.tile([P, HC, K], F16)
            nc.vector.tensor_tensor(sqk[:], ek[0][:, SIN], ek[0][:, SIN], ALU.mult)
            nc.vector.tensor_scalar(tk[:], sqk[:], -4.0, 2.0, ALU.mult, ALU.add)
            tkb = tk[:, None, :, :].to_broadcast((P, 1, HC, K))

            # k j=1: fused (t+-1)*e0, cos first (score stationary order)
            nc.vector.scalar_tensor_tensor(ek[1][:, COS:COS + 1], tkb, -1.0,
                                           ek[0][:, COS:COS + 1], ALU.add, ALU.mult)
            nc.vector.scalar_tensor_tensor(ek[1][:, SIN:SIN + 1], tkb, 1.0,
                                           ek[0][:, SIN:SIN + 1], ALU.add, ALU.mult)

            # k j=2 stacked
            uk = scr.tile([P, 2, HC, K], F16, tag="uk", bufs=2)
            nc.vector.tensor_tensor(uk[:], tk[:, None, :, :].to_broadcast(
                (P, 2, HC, K)), ek[1][:], ALU.mult)
            nc.vector.tensor_tensor(ek[2][:], uk[:], ek[0][:], ALU.subtract)

            uq = scr.tile([P, 2, HC, QS], F16, tag="uq", bufs=2)
            nc.vector.tensor_tensor(uq[:], tqb2, eq[1][:], ALU.mult)
            nc.vector.tensor_tensor(eq[2][:], uq[:], eq[0][:], ALU.subtract)
            fold(2)
            nc.vector.tensor_tensor(uq[:], tqb2, eq[2][:], ALU.mult)
            nc.vector.tensor_tensor(eq[3][:], uq[:], eq[1][:], ALU.subtract)
            fold(3)
            # negated j=3 fold for the PE-absorbed k j=3 subtract
            naq3 = chain.tile([P, 2, HC, QS], F16)
            nc.vector.tensor_scalar_mul(naq3[:], eq[3][:],
                                        -float(A_COEF[3] / A_COEF[0]))

            # value arrives mid-kernel on the scalar queue: tile_wait_until
            # stops the scheduler from hoisting its desc-gen into the input
            # window (observed: 512KB of v landing at t=10us starves the
            # critical qT/wq/kT/wk loads)
            v16 = const.tile([P, KC, DV], BF16)
            with tc.tile_wait_until(0.018):
                nc.scalar.dma_start(v16[:], v_in)
            # preload the exp activation table during the score phase; the
            # input slice is one the LAST sin wrote so the scheduler cannot
            # hoist it before the sins (hoisting forces TWO extra ~1.3us
            # table loads: sin -> exp -> sin again)
            dummy = const.tile([P, 1], F32)
            nc.scalar.activation(dummy[:], ek[0][:, COS, HC - 1, 0:1], AF.Exp)

        with tc.tile_pool(name="ps_score", bufs=1, space="PSUM") as ps_score, \
             tc.tile_pool(name="ps_tail", bufs=1, space="PSUM") as ps_tail:
            score_ps = [ps_score.tile([P, QS], F32, name=f"score_{kc}")
                        for kc in range(KC)]

            # score matmuls: contract h; half=0 uses cos_k stationary x sin_q
            # moving, half=1 sin_k x cos_q.
            def score_mm(j, hs, half, kc):
                nc.tensor.matmul(
                    score_ps[kc][:, :],
                    ek[j][:, 1 - half, hs, kc * P:(kc + 1) * P],
                    aq[j][:, half, hs, :],
                    start=(j == 0 and hs == 0 and half == 0),
                    stop=(j == M - 1 and hs == HC - 1 and half == 1))

            for j in range(M - 1):
                for half in range(2):
                    for hs in range(HC):
                        for kc in range(KC):
                            score_mm(j, hs, half, kc)

            # Last harmonic runs kc-major; the UNNORMALIZED bf16 exp feeds
            # the context matmuls (transposed [dv,q] layout, v stationary),
            # and the rank-1 row-sum, software-pipelined one kc behind the
            # score matmuls.  PSUM banks: score 4 + sums 1 + ctxT 3 = 8
            # (dv chunks 2+3 share one bank: the start=True of the first
            # group zeroes the bank; the second group starts with
            # has_written=0 so its first matmul overwrites).
            exp_bf = tailp.tile([P, KC, QS], BF16)
            sums_ps = ps_tail.tile([P, QS], F32, tag="sums")
            pscA = ps_tail.tile([P, QS], F32, tag="ctxA", name="ctxA")
            pscB = ps_tail.tile([P, QS], F32, tag="ctxB", name="ctxB")
            pscCD = ps_tail.tile([P, 2, QS], F32, tag="ctxCD", name="ctxCD")

            def ctx_mm(kc, dc):
                out = (pscA[:, :] if dc == 0 else pscB[:, :]
                       if dc == 1 else pscCD[:, dc - 2, :])
                nc.tensor.matmul(out, v16[:, kc, dc * P:(dc + 1) * P],
                                 exp_bf[:, kc, :],
                                 start=(kc == 0 and dc != 3), stop=(kc == KC - 1),
                                 skip_group_check=(dc >= 2))

            # k j=3 multiply runs here, split per key-chunk on the DVE; its
            # subtract is absorbed into the score matmuls below, so each
            # chunk's last-harmonic matmuls + exp pipeline into the chain.
            uk3s = {}

            def kj3_chunk(kc):
                sl = slice(kc * P, (kc + 1) * P)
                uk3s[kc] = scr.tile([P, 2, HC, P], F16, tag="uk3", bufs=2,
                                    name=f"uk3_{kc}")
                nc.vector.tensor_tensor(
                    uk3s[kc][:], tk[:, None, :, sl].to_broadcast((P, 2, HC, P)),
                    ek[2][:, :, :, sl], ALU.mult)

            def lastj_mms(kc):
                for half in range(2):
                    for hs in range(HC):
                        nc.tensor.matmul(
                            score_ps[kc][:, :],
                            uk3s[kc][:, 1 - half, hs, :],
                            aq[3][:, half, hs, :], start=False, stop=False)
                        nc.tensor.matmul(
                            score_ps[kc][:, :],
                            ek[1][:, 1 - half, hs, kc * P:(kc + 1) * P],
                            naq3[:, half, hs, :], start=False,
                            stop=(hs == HC - 1 and half == 1))

            # Close ALL four score groups first (they are only DVE-paced),
            # so the four exps fire back-to-back; sums matmuls ride right
            # behind each exp (they gate the reciprocal); the context
            # matmuls are off the critical path and go last.  The previous
            # interleaving queued lastj(3) behind exp-gated tail matmuls,
            # serializing an exp<->PE ping-pong that cost ~2us.
            for kc in range(KC):
                kj3_chunk(kc)
                lastj_mms(kc)
            for kc in range(KC):
                nc.scalar.activation(exp_bf[:, kc, :], score_ps[kc][:, :], AF.Exp)
                nc.tensor.matmul(sums_ps[0:1, :], ones_bf[:], exp_bf[:, kc, :],
                                 start=(kc == 0), stop=(kc == KC - 1))
            for kc in range(KC):
                for dc in range(4):
                    ctx_mm(kc, dc)

            # one reciprocal of the row sums (read straight from PSUM);
            # rank-1 PE broadcast across partitions; SBUF fp16 copy so the
            # normalizes run in 2x mode.
            rec32 = tailp.tile([1, QS], F32)
            nc.vector.reciprocal_approx_fast(rec32[:], sums_ps[0:1, :])
            rec16 = tailp.tile([1, QS], F16)
            nc.vector.tensor_copy(rec16[:], rec32[:])
            nc.tensor.matmul(sums_ps[:, :], ones16[:], rec16[:], start=True,
                             stop=True, skip_group_check=True)
            bc16 = tailp.tile([P, QS], F16)
            nc.vector.tensor_copy(bc16[:], sums_ps[:, :])

            # attn path: normalize bf16 (2x) + DMA in kc-pair halves
            attnT = tailp.tile([P, KC, QS], BF16)
            for h2 in range(2):
                sl = slice(h2 * 2, h2 * 2 + 2)
                nc.vector.tensor_tensor(
                    attnT[:, sl, :], exp_bf[:, sl, :],
                    bc16[:, None, :].to_broadcast((P, 2, QS)), ALU.mult)
                nc.sync.dma_start(attnT_out[:, sl, :], attnT[:, sl, :])

            # ctxT path: normalize along q by the same broadcast, out bf16
            ctxT = tailp.tile([P, 4, QS], BF16)
            nc.vector.tensor_tensor(ctxT[:, 0, :], pscA[:, :], bc16[:], ALU.mult)
            nc.vector.tensor_tensor(ctxT[:, 1, :], pscB[:, :], bc16[:], ALU.mult)
            nc.vector.tensor_tensor(
                ctxT[:, 2:4, :], pscCD[:, :, :],
                bc16[:, None, :].to_broadcast((P, 2, QS)), ALU.mult)
            nc.sync.dma_start(ctxT_out[:], ctxT[:])


def build_nc():
    nc = bacc.Bacc("TRN2", target_bir_lowering=False, debug=False)
    ins = [
        nc.dram_tensor("qT", [P, DC, QS], F16, kind="ExternalInput").ap(),
        nc.dram_tensor("kT", [P, DC, K], F16, kind="ExternalInput").ap(),
        nc.dram_tensor("value", [P, KC, DV], BF16, kind="ExternalInput").ap(),
        nc.dram_tensor("Wq", [P, DC, H], F16, kind="ExternalInput").ap(),
        nc.dram_tensor("Wk", [P, DC, H], F16, kind="ExternalInput").ap(),
        nc.dram_tensor("consts", [P, 5, HC], F32, kind="ExternalInput").ap(),
    ]
    outs = [
        nc.dram_tensor("attnT", [P, KC, QS], BF16, kind="ExternalOutput").ap(),
        nc.dram_tensor("ctxT", [P, 4, QS], BF16, kind="ExternalOutput").ap(),
    ]
    with tile.TileContext(nc) as tc:
        _build_tile_kernel(tc, ins, outs)
    nc.compile()
    return nc


_NC_CACHE = None


def _get_nc():
    global _NC_CACHE
    if _NC_CACHE is None:
        _NC_CACHE = build_nc()
    return _NC_CACHE


def _part3(a, chunks):
    """[chunks*128, N] row-major -> [128, chunks, N] (partition-major)."""
    n = a.shape[-1]
    return np.ascontiguousarray(a.reshape(chunks, P, n).transpose(1, 0, 2))


def make_in_maps(query, key, value, Wq, bq, Wk, bk, wv):
    wq16 = _part3(Wq, DC).astype(np.float16)
    wk16 = _part3(Wk, DC).astype(np.float16)
    consts_rows = np.stack([
        OM0 * bq, HALF_PI - OM0 * bq, OM0 * bk, HALF_PI - OM0 * bk,
        A_COEF[0] * wv,
    ]).astype(np.float32)                       # [5, H]
    consts = np.ascontiguousarray(
        consts_rows.reshape(5, HC, P).transpose(2, 0, 1))   # [P, 5, HC]
    in_maps = []
    for c in range(N_CORES):
        b, half = c // 2, c % 2
        qs = half * QS
        in_maps.append({
            "qT": _part3(query[b, qs:qs + QS, :].T, DC).astype(np.float16),
            "kT": _part3(key[b].T, DC).astype(np.float16),
            "value": _part3(value[b], KC).astype(ml_dtypes.bfloat16),
            "Wq": wq16,
            "Wk": wk16,
            "consts": consts,
        })
    return in_maps


def gather_results(results):
    context = np.empty((B, Q, DV), np.float32)
    attn = np.empty((B, Q, K), np.float32)
    for c, r in enumerate(results):
        b, half = c // 2, c % 2
        qs = half * QS
        # attnT [P, KC, QS]: attn[q, kc*128+p] = attnT[p, kc, q]
        a = np.asarray(r["attnT"]).astype(np.float32)       # [P, KC, QS]
        attn[b, qs:qs + QS, :] = a.transpose(2, 1, 0).reshape(QS, K)
        ct = np.asarray(r["ctxT"]).astype(np.float32)       # [P, 4, QS]
        context[b, qs:qs + QS, :] = ct.transpose(2, 1, 0).reshape(QS, DV)
    return context, attn


def kernel(query, key, value, Wq, bq, Wk, bk, wv, bv, **run_kwargs):
    nc = _get_nc()
    in_maps = make_in_maps(
        np.asarray(query, np.float32), np.asarray(key, np.float32),
        np.asarray(value, np.float32), np.asarray(Wq, np.float32),
        np.asarray(bq, np.float32), np.asarray(Wk, np.float32),
        np.asarray(bk, np.float32), np.asarray(wv, np.float32))
    res = run_bass_kernel_spmd(nc, in_maps, core_ids=list(range(N_CORES)),
                               **run_kwargs)
    out = gather_results(res.results)
    if run_kwargs:
        return out, res
    return out
